# revision 4
# baseline (speedup 1.0000x reference)
"""Chamfer-augmented kernel for Trainium2 (8 NeuronCores, data-parallel over batch).

For each batch b and each grid sample s:
    mins[s]  = min_j ||grid_s - pred_j||
    mins2[s] = min_j ||grid_s - gt_j||
    out[b]   = mean_s |mins - mins2|

Per-core algorithm (batch b on core b):
  PSUM holds d^2(s,j) = x_s^2 + q_j - 2 x_s . y_j directly: a single K=21 bf16
  matmul per 512-col chunk using exact Karatsuba splits (x = xh+xl, y' = -2y =
  yh+yl, q = qh+ql per coordinate, x^2 = x2h+x2m+x2l):
    lhsT rows: [xh]*3 [xh]*3 [xl]*3 [xl]*3 [1]*6 [x2h x2m x2l]
    rhs  rows: [yh]*3 [yl]*3 [yh]*3 [yl]*3 [qh]*3 [ql]*3 [1]*3
  Evacuation never materializes the distance matrix: per m-tile (128 samples),
  8192 columns stream through an 8-bank PSUM ring as two 2048-col groups that
  ScalarE converts to f16 (CC) and four 1024-col groups that VectorE consumes
  with fused running-min scans:
    tensor_tensor_scan(out, data0=PSUM_f32, data1=CC_f16, init=chain,
                       op0=min, op1=min)
  Each scan first-touches 1 PSUM + 1 CC element per cycle, and the chain's
  initial value threads the running min across the four scans, so the m-tile
  min falls out of the last scan's final column with no separate fold tree.
"""

import os

import numpy as np

import concourse.bass as bass
import concourse.tile as tile
from concourse import bacc, mybir, bass_utils

F32 = mybir.dt.float32
BF16 = mybir.dt.bfloat16
F16 = mybir.dt.float16
AX = mybir.AxisListType
OP = mybir.AluOpType
AF = mybir.ActivationFunctionType

BS = 8
S = 2048          # n_samples (grid points)
J = 8192          # n_points (preds/gts)
NM = S // 128     # 16 m-tiles
PACK = 8          # prep packing for rhs: [3*PACK, J/PACK]
JP = J // PACK    # 1024
GPACK = 8         # prep packing for grid: [3*GPACK, S/GPACK]
SP = S // GPACK   # 256

# lhsT/rhs row layout (K = 21)
#   rows 0-2   lhsT xh_c        rhs yh_c
#   rows 3-5   lhsT xh_c        rhs yl_c
#   rows 6-8   lhsT xl_c        rhs yh_c
#   rows 9-11  lhsT xl_c        rhs yl_c
#   rows 12-14 lhsT ones        rhs qh_c
#   rows 15-17 lhsT ones        rhs ql_c
#   rows 18-20 lhsT x2h/m/l     rhs ones
K = 21


def _build_rhs(nc, sb, pts_dram, name):
    """Load one point set (packed [24, 1024] f32) and build the [21, J] bf16 rhs."""
    Y = sb.tile([3 * PACK, JP], F32, tag=f"y_{name}")
    nc.sync.dma_start(Y[:], pts_dram)
    # q = y^2 per coordinate (ScalarE), yh = bf16(-2y) (ScalarE)
    SQ = sb.tile([3 * PACK, JP], F32, tag=f"sq_{name}")
    nc.scalar.activation(SQ[:], Y[:], AF.Square)
    YH = sb.tile([3 * PACK, JP], BF16, tag=f"yh_{name}")
    nc.scalar.activation(YH[:], Y[:], AF.Copy, scale=-2.0)
    # yl = (-2y) - yh (VectorE), qh = bf16(q) (ScalarE), ql = q - qh (VectorE)
    YL = sb.tile([3 * PACK, JP], BF16, tag=f"yl_{name}")
    nc.vector.scalar_tensor_tensor(YL[:], Y[:], -2.0, YH[:], op0=OP.mult, op1=OP.subtract)
    QH = sb.tile([3 * PACK, JP], BF16, tag=f"qh_{name}")
    nc.scalar.activation(QH[:], SQ[:], AF.Copy)
    QL = sb.tile([3 * PACK, JP], BF16, tag=f"ql_{name}")
    nc.vector.tensor_tensor(QL[:], SQ[:], QH[:], op=OP.subtract)
    ONESJ = sb.tile([3 * PACK, JP], BF16, tag=f"onesj_{name}")
    nc.gpsimd.memset(ONESJ[:], 1.0)

    RH = sb.tile([K, J], BF16, tag=f"rh_{name}")
    # packed [24, 1024] -> [3, 8192] row groups; AP iteration orders match.
    for r0, src in ((0, YH), (3, YL), (6, YH), (9, YL), (12, QH), (15, QL), (18, ONESJ)):
        nc.sync.dma_start(RH[r0:r0 + 3, :], src[:])
    return RH


def _build_lhs(nc, sb, ps_s, grid_dram, sel_dram):
    """Build the [21, S] bf16 lhsT from the packed grid [24, 256]."""
    GP = sb.tile([3 * GPACK, SP], F32, tag="gp")
    nc.sync.dma_start(GP[:], grid_dram)
    SEL = sb.tile([3 * GPACK, GPACK], F32, tag="sel")
    nc.sync.dma_start(SEL[:], sel_dram)

    XH = sb.tile([3 * GPACK, SP], BF16, tag="xh")
    nc.scalar.activation(XH[:], GP[:], AF.Copy)
    XL = sb.tile([3 * GPACK, SP], BF16, tag="xl")
    nc.vector.tensor_tensor(XL[:], GP[:], XH[:], op=OP.subtract)
    SQG = sb.tile([3 * GPACK, SP], F32, tag="sqg")
    nc.scalar.activation(SQG[:], GP[:], AF.Square)
    # x^2 = sum over coords via selector matmul: [24,8].T @ [24,256] -> [8,256]
    PX = ps_s.tile([128, 1024], F32, tag="psc")
    X2P = PX[0:GPACK, 0:SP]
    nc.tensor.matmul(X2P, SEL[:], SQG[:], start=True, stop=True)
    X2S = sb.tile([GPACK, SP], F32, tag="x2s")
    nc.scalar.activation(X2S[:], X2P, AF.Copy)
    # three-term bf16 split of x^2, each packed [8, 256]
    X2H = sb.tile([GPACK, SP], BF16, tag="x2h")
    nc.scalar.activation(X2H[:], X2S[:], AF.Copy)
    R1 = sb.tile([GPACK, SP], F32, tag="x2r1")
    nc.vector.tensor_tensor(R1[:], X2S[:], X2H[:], op=OP.subtract)
    X2M = sb.tile([GPACK, SP], BF16, tag="x2m")
    nc.scalar.activation(X2M[:], R1[:], AF.Copy)
    R2 = sb.tile([GPACK, SP], F32, tag="x2r2")
    nc.vector.tensor_tensor(R2[:], R1[:], X2M[:], op=OP.subtract)
    X2L = sb.tile([GPACK, SP], BF16, tag="x2l")
    nc.scalar.activation(X2L[:], R2[:], AF.Copy)
    ONESS = sb.tile([3 * GPACK, SP], BF16, tag="oness")
    nc.gpsimd.memset(ONESS[:], 1.0)

    LH = sb.tile([K, S], BF16, tag="lh")
    for r0, src in ((0, XH), (3, XH), (6, XL), (9, XL), (12, ONESS)):
        nc.sync.dma_start(LH[r0:r0 + 3, :], src[:])
    nc.sync.dma_start(LH[15:18, :], ONESS[:])
    for r, src in ((18, X2H), (19, X2M), (20, X2L)):
        nc.sync.dma_start(LH[r:r + 1, :], src[:])
    return LH


def _minloop(nc, wk, ps_a, ps_s, LH, RH, MINS, INF):
    """Per m-tile: 2 act groups of 2048 (ScalarE -> f16 CC) and 4 chained
    1024-col running-min scans (VectorE) pairing fresh PSUM with CC."""
    for m in range(NM):
        LHm = LH[:, m * 128:(m + 1) * 128]
        prev = INF[:]
        O = None
        for h in range(2):  # half = [act 2048 | scan 1024 | scan 1024]
            base = h * 4096
            PA = ps_a.tile([128, 2048], F32, tag="pa")
            for t in range(4):
                nc.tensor.matmul(PA[:, t * 512:(t + 1) * 512], LHm,
                                 RH[:, base + t * 512:base + (t + 1) * 512],
                                 start=True, stop=True)
            CC = wk.tile([128, 2048], F16, tag="cc")
            nc.scalar.activation(CC[:], PA[:], AF.Copy)
            for u in range(2):
                PS = ps_s.tile([128, 1024], F32, tag="psc")
                j0 = base + 2048 + u * 1024
                for t in range(2):
                    nc.tensor.matmul(PS[:, t * 512:(t + 1) * 512], LHm,
                                     RH[:, j0 + t * 512:j0 + (t + 1) * 512],
                                     start=True, stop=True)
                O = wk.tile([128, 1024], F16, tag="so")
                nc.vector.tensor_tensor_scan(O[:], PS[:], CC[:, u * 1024:(u + 1) * 1024],
                                             prev, op0=OP.min, op1=OP.min)
                prev = O[:, 1023:1024]
        # m-tile min = last scan's final running value (ScalarE copy; S has slack)
        nc.scalar.activation(MINS[:, m:m + 1], O[:, 1023:1024], AF.Copy)


def _build_module():
    nc = bacc.Bacc("TRN2", target_bir_lowering=False, debug=False, num_devices=BS)
    grid_p = nc.dram_tensor("grid_p", [3 * GPACK, SP], F32, kind="ExternalInput").ap()
    sel24 = nc.dram_tensor("sel24", [3 * GPACK, GPACK], F32, kind="ExternalInput").ap()
    preds_p = nc.dram_tensor("preds_p", [3 * PACK, JP], F32, kind="ExternalInput").ap()
    gts_p = nc.dram_tensor("gts_p", [3 * PACK, JP], F32, kind="ExternalInput").ap()
    out_d = nc.dram_tensor("out", [1, 1], F32, kind="ExternalOutput").ap()

    with tile.TileContext(nc) as tc:
        with tc.tile_pool(name="sb", bufs=1) as sb, \
             tc.tile_pool(name="wk", bufs=2) as wk, \
             tc.tile_pool(name="ps_a", bufs=1, space="PSUM") as ps_a, \
             tc.tile_pool(name="ps_s", bufs=2, space="PSUM") as ps_s:
            LH = _build_lhs(nc, sb, ps_s, grid_p, sel24)
            RHP = _build_rhs(nc, sb, preds_p, "p")
            RHG = _build_rhs(nc, sb, gts_p, "g")

            INF = sb.tile([128, 1], F32, tag="inf")
            nc.vector.memset(INF[:], 3.0e38)

            MINS_P = sb.tile([128, NM], F32, tag="minsp")
            MINS_G = sb.tile([128, NM], F32, tag="minsg")
            _minloop(nc, wk, ps_a, ps_s, LH, RHP, MINS_P, INF)
            _minloop(nc, wk, ps_a, ps_s, LH, RHG, MINS_G, INF)

            # d = sqrt(max(d^2, eps)) with one Newton refinement step
            def _distances(MINS, tag):
                D2 = sb.tile([128, NM], F32, tag=f"d2{tag}")
                nc.vector.tensor_scalar_max(D2[:], MINS[:], 1e-12)
                D0 = sb.tile([128, NM], F32, tag=f"d0{tag}")
                nc.scalar.activation(D0[:], D2[:], AF.Sqrt)
                R = sb.tile([128, NM], F32, tag=f"r{tag}")
                nc.vector.reciprocal(R[:], D0[:])
                D1 = sb.tile([128, NM], F32, tag=f"d1{tag}")
                nc.vector.tensor_tensor(D1[:], D2[:], R[:], op=OP.mult)
                nc.vector.tensor_tensor(D1[:], D1[:], D0[:], op=OP.add)
                nc.vector.tensor_scalar_mul(D1[:], D1[:], 0.5)
                return D1

            DP = _distances(MINS_P, "dp")
            DG = _distances(MINS_G, "dg")

            # mean_s |dp - dg|
            DIFF = sb.tile([128, NM], F32, tag="diff")
            nc.vector.tensor_tensor(DIFF[:], DP[:], DG[:], op=OP.subtract)
            SROW = sb.tile([128, 1], F32, tag="srow")
            nc.vector.tensor_reduce(SROW[:], DIFF[:], axis=AX.X, op=OP.add,
                                    apply_absolute_value=True)
            ONE32 = sb.tile([128, 1], F32, tag="one32")
            nc.vector.memset(ONE32[:], 1.0)
            PGX = ps_a.tile([128, 2048], F32, tag="pa")
            TOT = PGX[0:1, 0:1]
            nc.tensor.matmul(TOT, ONE32[:], SROW[:], start=True, stop=True)
            OUT = sb.tile([1, 1], F32, tag="outsb")
            nc.scalar.activation(OUT[:], TOT, AF.Copy, scale=1.0 / float(S))
            nc.sync.dma_start(out_d, OUT[:])
    nc.compile()
    return nc


_NC = None


def _get_nc():
    global _NC
    if _NC is None:
        _NC = _build_module()
    return _NC


_SEL24 = np.zeros((3 * GPACK, GPACK), np.float32)
for _r in range(3 * GPACK):
    _SEL24[_r, _r % GPACK] = 1.0


def _in_maps(gts, preds, grid_points):
    maps = []
    for b in range(BS):
        g = np.ascontiguousarray(grid_points[b], np.float32)
        maps.append({
            "grid_p": np.ascontiguousarray(g.T.reshape(3 * GPACK, SP)),
            "sel24": _SEL24,
            "preds_p": np.ascontiguousarray(preds[b], np.float32).T.reshape(3 * PACK, JP).copy(),
            "gts_p": np.ascontiguousarray(gts[b], np.float32).T.reshape(3 * PACK, JP).copy(),
        })
    return maps


def kernel(gts, preds, grid_points, _trace=False, _trace_kwargs=None):
    nc = _get_nc()
    res = bass_utils.run_bass_kernel_spmd(
        nc, _in_maps(gts, preds, grid_points), core_ids=list(range(BS)),
        trace=_trace, **(_trace_kwargs or {}))
    out = np.array([res.results[b]["out"][0, 0] for b in range(BS)], np.float32)
    if _trace:
        return out, res
    return out


# revision 6
# speedup vs baseline: 1.0775x; 1.0775x over previous
"""Chamfer-augmented kernel for Trainium2 (8 NeuronCores, data-parallel over batch).

For each batch b and each grid sample s:
    mins[s]  = min_j ||grid_s - pred_j||
    mins2[s] = min_j ||grid_s - gt_j||
    out[b]   = mean_s |mins - mins2|

Per-core algorithm (batch b on core b):
  PSUM holds d^2(s,j) = x_s^2 + q_j - 2 x_s . y_j directly: a single K=21 bf16
  matmul per 512-col chunk using exact Karatsuba splits (x = xh+xl, y' = -2y =
  yh+yl, q = qh+ql per coordinate, x^2 = x2h+x2m+x2l):
    lhsT rows: [xh]*3 [xh]*3 [xl]*3 [xl]*3 [1]*6 [x2h x2m x2l]
    rhs  rows: [yh]*3 [yl]*3 [yh]*3 [yl]*3 [qh]*3 [ql]*3 [1]*3
  Evacuation never materializes the distance matrix: per m-tile (128 samples),
  8192 columns stream through an 8-bank PSUM ring as two 2048-col groups that
  ScalarE converts to f16 (CC) and four 1024-col groups that VectorE consumes
  with fused running-min scans:
    tensor_tensor_scan(out, data0=PSUM_f32, data1=CC_f16, init=chain,
                       op0=min, op1=min)
  Each scan first-touches 1 PSUM + 1 CC element per cycle, and the chain's
  initial value threads the running min across the four scans, so the m-tile
  min falls out of the last scan's final column with no separate fold tree.
"""

import os

import numpy as np

import concourse.bass as bass
import concourse.tile as tile
from concourse import bacc, mybir, bass_utils

F32 = mybir.dt.float32
BF16 = mybir.dt.bfloat16
F16 = mybir.dt.float16
AX = mybir.AxisListType
OP = mybir.AluOpType
AF = mybir.ActivationFunctionType

BS = 8
S = 2048          # n_samples (grid points)
J = 8192          # n_points (preds/gts)
NM = S // 128     # 16 m-tiles
PACK = 8          # prep packing for rhs: [3*PACK, J/PACK]
JP = J // PACK    # 1024
GPACK = 8         # prep packing for grid: [3*GPACK, S/GPACK]
SP = S // GPACK   # 256

# lhsT/rhs row layout (K = 21)
#   rows 0-2   lhsT xh_c        rhs yh_c
#   rows 3-5   lhsT xh_c        rhs yl_c
#   rows 6-8   lhsT xl_c        rhs yh_c
#   rows 9-11  lhsT xl_c        rhs yl_c
#   rows 12-14 lhsT ones        rhs qh_c
#   rows 15-17 lhsT ones        rhs ql_c
#   rows 18-20 lhsT x2h/m/l     rhs ones
K = 21


def _build_rhs(nc, sb, pts_dram, name):
    """Load one point set (packed [24, 1024] f32) and build the [21, J] bf16 rhs."""
    Y = sb.tile([3 * PACK, JP], F32, tag=f"y_{name}")
    nc.sync.dma_start(Y[:], pts_dram)
    # q = y^2 per coordinate (ScalarE), yh = bf16(-2y) (ScalarE)
    SQ = sb.tile([3 * PACK, JP], F32, tag=f"sq_{name}")
    nc.scalar.activation(SQ[:], Y[:], AF.Square)
    YH = sb.tile([3 * PACK, JP], BF16, tag=f"yh_{name}")
    nc.scalar.activation(YH[:], Y[:], AF.Copy, scale=-2.0)
    # yl = (-2y) - yh (VectorE), qh = bf16(q) (ScalarE), ql = q - qh (VectorE)
    YL = sb.tile([3 * PACK, JP], BF16, tag=f"yl_{name}")
    nc.vector.scalar_tensor_tensor(YL[:], Y[:], -2.0, YH[:], op0=OP.mult, op1=OP.subtract)
    QH = sb.tile([3 * PACK, JP], BF16, tag=f"qh_{name}")
    nc.scalar.activation(QH[:], SQ[:], AF.Copy)
    QL = sb.tile([3 * PACK, JP], BF16, tag=f"ql_{name}")
    nc.vector.tensor_tensor(QL[:], SQ[:], QH[:], op=OP.subtract)
    ONESJ = sb.tile([3 * PACK, JP], BF16, tag=f"onesj_{name}")
    nc.gpsimd.memset(ONESJ[:], 1.0)

    RH = sb.tile([K, J], BF16, tag=f"rh_{name}")
    # packed [24, 1024] -> [3, 8192] row groups; AP iteration orders match.
    for r0, src in ((0, YH), (3, YL), (6, YH), (9, YL), (12, QH), (15, QL), (18, ONESJ)):
        nc.sync.dma_start(RH[r0:r0 + 3, :], src[:])
    return RH


def _build_lhs(nc, sb, ps_s, grid_dram, sel_dram):
    """Build the [21, S] bf16 lhsT from the packed grid [24, 256]."""
    GP = sb.tile([3 * GPACK, SP], F32, tag="gp")
    nc.sync.dma_start(GP[:], grid_dram)
    SEL = sb.tile([3 * GPACK, GPACK], F32, tag="sel")
    nc.sync.dma_start(SEL[:], sel_dram)

    XH = sb.tile([3 * GPACK, SP], BF16, tag="xh")
    nc.scalar.activation(XH[:], GP[:], AF.Copy)
    XL = sb.tile([3 * GPACK, SP], BF16, tag="xl")
    nc.vector.tensor_tensor(XL[:], GP[:], XH[:], op=OP.subtract)
    SQG = sb.tile([3 * GPACK, SP], F32, tag="sqg")
    nc.scalar.activation(SQG[:], GP[:], AF.Square)
    # x^2 = sum over coords via selector matmul: [24,8].T @ [24,256] -> [8,256]
    PX = ps_s.tile([128, 1024], F32, tag="psc")
    X2P = PX[0:GPACK, 0:SP]
    nc.tensor.matmul(X2P, SEL[:], SQG[:], start=True, stop=True)
    X2S = sb.tile([GPACK, SP], F32, tag="x2s")
    nc.scalar.activation(X2S[:], X2P, AF.Copy)
    # three-term bf16 split of x^2, each packed [8, 256]
    X2H = sb.tile([GPACK, SP], BF16, tag="x2h")
    nc.scalar.activation(X2H[:], X2S[:], AF.Copy)
    R1 = sb.tile([GPACK, SP], F32, tag="x2r1")
    nc.vector.tensor_tensor(R1[:], X2S[:], X2H[:], op=OP.subtract)
    X2M = sb.tile([GPACK, SP], BF16, tag="x2m")
    nc.scalar.activation(X2M[:], R1[:], AF.Copy)
    R2 = sb.tile([GPACK, SP], F32, tag="x2r2")
    nc.vector.tensor_tensor(R2[:], R1[:], X2M[:], op=OP.subtract)
    X2L = sb.tile([GPACK, SP], BF16, tag="x2l")
    nc.scalar.activation(X2L[:], R2[:], AF.Copy)
    ONESS = sb.tile([3 * GPACK, SP], BF16, tag="oness")
    nc.gpsimd.memset(ONESS[:], 1.0)

    LH = sb.tile([K, S], BF16, tag="lh")
    for r0, src in ((0, XH), (3, XH), (6, XL), (9, XL), (12, ONESS)):
        nc.sync.dma_start(LH[r0:r0 + 3, :], src[:])
    nc.sync.dma_start(LH[15:18, :], ONESS[:])
    for r, src in ((18, X2H), (19, X2M), (20, X2L)):
        nc.sync.dma_start(LH[r:r + 1, :], src[:])
    return LH


def _minloop(nc, wk, ps_a, ps_s, LH, RH, MINS, INF):
    """Per m-tile: 4 act groups of 1024 (ScalarE -> f16 CC) and 4 chained
    1024-col running-min scans (VectorE) pairing fresh PSUM with CC.
    Both PSUM tags are double-buffered (8 banks total) so each group's
    matmuls prefill while the previous group is consumed."""
    for m in range(NM):
        LHm = LH[:, m * 128:(m + 1) * 128]
        prev = INF[:]
        O = None
        for u in range(4):  # unit = [act 1024 | scan 1024]
            PA = ps_a.tile([128, 1024], F32, tag="pa")
            base = u * 2048
            for t in range(2):
                nc.tensor.matmul(PA[:, t * 512:(t + 1) * 512], LHm,
                                 RH[:, base + t * 512:base + (t + 1) * 512],
                                 start=True, stop=True)
            CC = wk.tile([128, 1024], F16, tag="cc")
            nc.scalar.activation(CC[:], PA[:], AF.Copy)
            PS = ps_s.tile([128, 1024], F32, tag="psc")
            for t in range(2):
                nc.tensor.matmul(PS[:, t * 512:(t + 1) * 512], LHm,
                                 RH[:, base + 1024 + t * 512:base + 1024 + (t + 1) * 512],
                                 start=True, stop=True)
            O = wk.tile([128, 1024], F16, tag="so")
            nc.vector.tensor_tensor_scan(O[:], PS[:], CC[:],
                                         prev, op0=OP.min, op1=OP.min)
            prev = O[:, 1023:1024]
        # m-tile min = last scan's final running value (ScalarE copy; S has slack)
        nc.scalar.activation(MINS[:, m:m + 1], O[:, 1023:1024], AF.Copy)


def _build_module():
    nc = bacc.Bacc("TRN2", target_bir_lowering=False, debug=False, num_devices=BS)
    grid_p = nc.dram_tensor("grid_p", [3 * GPACK, SP], F32, kind="ExternalInput").ap()
    sel24 = nc.dram_tensor("sel24", [3 * GPACK, GPACK], F32, kind="ExternalInput").ap()
    preds_p = nc.dram_tensor("preds_p", [3 * PACK, JP], F32, kind="ExternalInput").ap()
    gts_p = nc.dram_tensor("gts_p", [3 * PACK, JP], F32, kind="ExternalInput").ap()
    out_d = nc.dram_tensor("out", [1, 1], F32, kind="ExternalOutput").ap()

    with tile.TileContext(nc) as tc:
        with tc.tile_pool(name="sb", bufs=1) as sb, \
             tc.tile_pool(name="wk", bufs=2) as wk, \
             tc.tile_pool(name="ps_a", bufs=2, space="PSUM") as ps_a, \
             tc.tile_pool(name="ps_s", bufs=2, space="PSUM") as ps_s:
            LH = _build_lhs(nc, sb, ps_s, grid_p, sel24)
            RHP = _build_rhs(nc, sb, preds_p, "p")
            RHG = _build_rhs(nc, sb, gts_p, "g")

            INF = sb.tile([128, 1], F32, tag="inf")
            nc.vector.memset(INF[:], 3.0e38)

            MINS_P = sb.tile([128, NM], F32, tag="minsp")
            MINS_G = sb.tile([128, NM], F32, tag="minsg")
            _minloop(nc, wk, ps_a, ps_s, LH, RHP, MINS_P, INF)
            _minloop(nc, wk, ps_a, ps_s, LH, RHG, MINS_G, INF)

            # d = sqrt(max(d^2, eps)) with one Newton refinement step
            def _distances(MINS, tag):
                D2 = sb.tile([128, NM], F32, tag=f"d2{tag}")
                nc.vector.tensor_scalar_max(D2[:], MINS[:], 1e-12)
                D0 = sb.tile([128, NM], F32, tag=f"d0{tag}")
                nc.scalar.activation(D0[:], D2[:], AF.Sqrt)
                R = sb.tile([128, NM], F32, tag=f"r{tag}")
                nc.vector.reciprocal(R[:], D0[:])
                D1 = sb.tile([128, NM], F32, tag=f"d1{tag}")
                nc.vector.tensor_tensor(D1[:], D2[:], R[:], op=OP.mult)
                nc.vector.tensor_tensor(D1[:], D1[:], D0[:], op=OP.add)
                nc.vector.tensor_scalar_mul(D1[:], D1[:], 0.5)
                return D1

            DP = _distances(MINS_P, "dp")
            DG = _distances(MINS_G, "dg")

            # mean_s |dp - dg|
            DIFF = sb.tile([128, NM], F32, tag="diff")
            nc.vector.tensor_tensor(DIFF[:], DP[:], DG[:], op=OP.subtract)
            SROW = sb.tile([128, 1], F32, tag="srow")
            nc.vector.tensor_reduce(SROW[:], DIFF[:], axis=AX.X, op=OP.add,
                                    apply_absolute_value=True)
            ONE32 = sb.tile([128, 1], F32, tag="one32")
            nc.vector.memset(ONE32[:], 1.0)
            PGX = ps_a.tile([128, 1024], F32, tag="pa")
            TOT = PGX[0:1, 0:1]
            nc.tensor.matmul(TOT, ONE32[:], SROW[:], start=True, stop=True)
            OUT = sb.tile([1, 1], F32, tag="outsb")
            nc.scalar.activation(OUT[:], TOT, AF.Copy, scale=1.0 / float(S))
            nc.sync.dma_start(out_d, OUT[:])
    nc.compile()
    return nc


_NC = None


def _get_nc():
    global _NC
    if _NC is None:
        _NC = _build_module()
    return _NC


_SEL24 = np.zeros((3 * GPACK, GPACK), np.float32)
for _r in range(3 * GPACK):
    _SEL24[_r, _r % GPACK] = 1.0


def _in_maps(gts, preds, grid_points):
    maps = []
    for b in range(BS):
        g = np.ascontiguousarray(grid_points[b], np.float32)
        maps.append({
            "grid_p": np.ascontiguousarray(g.T.reshape(3 * GPACK, SP)),
            "sel24": _SEL24,
            "preds_p": np.ascontiguousarray(preds[b], np.float32).T.reshape(3 * PACK, JP).copy(),
            "gts_p": np.ascontiguousarray(gts[b], np.float32).T.reshape(3 * PACK, JP).copy(),
        })
    return maps


def kernel(gts, preds, grid_points, _trace=False, _trace_kwargs=None):
    nc = _get_nc()
    res = bass_utils.run_bass_kernel_spmd(
        nc, _in_maps(gts, preds, grid_points), core_ids=list(range(BS)),
        trace=_trace, **(_trace_kwargs or {}))
    out = np.array([res.results[b]["out"][0, 0] for b in range(BS)], np.float32)
    if _trace:
        return out, res
    return out


# revision 8
# speedup vs baseline: 1.1932x; 1.1074x over previous
"""Chamfer-augmented kernel for Trainium2 (8 NeuronCores, data-parallel over batch).

For each batch b and each grid sample s:
    mins[s]  = min_j ||grid_s - pred_j||
    mins2[s] = min_j ||grid_s - gt_j||
    out[b]   = mean_s |mins - mins2|

Per-core algorithm (batch b on core b):
  PSUM holds d^2(s,j) = x_s^2 + q_j - 2 x_s . y_j directly: a single K=21 bf16
  matmul per 512-col chunk using exact Karatsuba splits (x = xh+xl, y' = -2y =
  yh+yl, q = qh+ql per coordinate, x^2 = x2h+x2m+x2l):
    lhsT rows: [xh]*3 [xh]*3 [xl]*3 [xl]*3 [1]*6 [x2h x2m x2l]
    rhs  rows: [yh]*3 [yl]*3 [yh]*3 [yl]*3 [qh]*3 [ql]*3 [1]*3
  Evacuation never materializes the distance matrix: per m-tile (128 samples),
  8192 columns stream through an 8-bank PSUM ring as two 2048-col groups that
  ScalarE converts to f16 (CC) and four 1024-col groups that VectorE consumes
  with fused running-min scans:
    tensor_tensor_scan(out, data0=PSUM_f32, data1=CC_f16, init=chain,
                       op0=min, op1=min)
  Each scan first-touches 1 PSUM + 1 CC element per cycle, and the chain's
  initial value threads the running min across the four scans, so the m-tile
  min falls out of the last scan's final column with no separate fold tree.
"""

import os

import numpy as np

import concourse.bass as bass
import concourse.tile as tile
from concourse import bacc, mybir, bass_utils

F32 = mybir.dt.float32
BF16 = mybir.dt.bfloat16
F16 = mybir.dt.float16
AX = mybir.AxisListType
OP = mybir.AluOpType
AF = mybir.ActivationFunctionType

BS = 8
S = 2048          # n_samples (grid points)
J = 8192          # n_points (preds/gts)
NM = S // 128     # 16 m-tiles
PACK = 8          # prep packing for rhs: [3*PACK, J/PACK]
JP = J // PACK    # 1024
GPACK = 8         # prep packing for grid: [3*GPACK, S/GPACK]
SP = S // GPACK   # 256

# lhsT/rhs row layout (K = 21)
#   rows 0-2   lhsT xh_c        rhs yh_c
#   rows 3-5   lhsT xh_c        rhs yl_c
#   rows 6-8   lhsT xl_c        rhs yh_c
#   rows 9-11  lhsT xl_c        rhs yl_c
#   rows 12-14 lhsT ones        rhs qh_c
#   rows 15-17 lhsT ones        rhs ql_c
#   rows 18-20 lhsT x2h/m/l     rhs ones
K = 21


def _build_rhs(nc, sb, pts_dram, name, dma):
    """Load one point set (packed [24, 1024] f32) and build the [21, J] bf16 rhs."""
    Y = sb.tile([3 * PACK, JP], F32, tag=f"y_{name}")
    dma(Y[:], pts_dram)
    # q = y^2 per coordinate (ScalarE), yh = bf16(-2y) (ScalarE)
    SQ = sb.tile([3 * PACK, JP], F32, tag=f"sq_{name}")
    nc.scalar.activation(SQ[:], Y[:], AF.Square)
    YH = sb.tile([3 * PACK, JP], BF16, tag=f"yh_{name}")
    nc.scalar.activation(YH[:], Y[:], AF.Copy, scale=-2.0)
    # yl = (-2y) - yh (VectorE), qh = bf16(q) (ScalarE), ql = q - qh (VectorE)
    YL = sb.tile([3 * PACK, JP], BF16, tag=f"yl_{name}")
    nc.vector.scalar_tensor_tensor(YL[:], Y[:], -2.0, YH[:], op0=OP.mult, op1=OP.subtract)
    QH = sb.tile([3 * PACK, JP], BF16, tag=f"qh_{name}")
    nc.scalar.activation(QH[:], SQ[:], AF.Copy)
    QL = sb.tile([3 * PACK, JP], BF16, tag=f"ql_{name}")
    nc.vector.tensor_tensor(QL[:], SQ[:], QH[:], op=OP.subtract)
    ONESJ = sb.tile([3 * PACK, JP], BF16, tag=f"onesj_{name}")
    nc.gpsimd.memset(ONESJ[:], 1.0)

    RH = sb.tile([K, J], BF16, tag=f"rh_{name}")
    # packed [24, 1024] -> [3, 8192] row groups; AP iteration orders match.
    for r0, src in ((0, YH), (3, YL), (6, YH), (9, YL), (12, QH), (15, QL), (18, ONESJ)):
        dma(RH[r0:r0 + 3, :], src[:])
    return RH


def _build_lhs(nc, sb, ps_s, grid_dram, sel_dram):
    """Build the [21, S] bf16 lhsT from the packed grid [24, 256]."""
    GP = sb.tile([3 * GPACK, SP], F32, tag="gp")
    nc.sync.dma_start(GP[:], grid_dram)
    SEL = sb.tile([3 * GPACK, GPACK], F32, tag="sel")
    nc.sync.dma_start(SEL[:], sel_dram)

    XH = sb.tile([3 * GPACK, SP], BF16, tag="xh")
    nc.scalar.activation(XH[:], GP[:], AF.Copy)
    XL = sb.tile([3 * GPACK, SP], BF16, tag="xl")
    nc.vector.tensor_tensor(XL[:], GP[:], XH[:], op=OP.subtract)
    SQG = sb.tile([3 * GPACK, SP], F32, tag="sqg")
    nc.scalar.activation(SQG[:], GP[:], AF.Square)
    # x^2 = sum over coords via selector matmul: [24,8].T @ [24,256] -> [8,256]
    PX = ps_s.tile([128, 1024], F32, tag="psc")
    X2P = PX[0:GPACK, 0:SP]
    nc.tensor.matmul(X2P, SEL[:], SQG[:], start=True, stop=True)
    X2S = sb.tile([GPACK, SP], F32, tag="x2s")
    nc.scalar.activation(X2S[:], X2P, AF.Copy)
    # three-term bf16 split of x^2, each packed [8, 256]
    X2H = sb.tile([GPACK, SP], BF16, tag="x2h")
    nc.scalar.activation(X2H[:], X2S[:], AF.Copy)
    R1 = sb.tile([GPACK, SP], F32, tag="x2r1")
    nc.vector.tensor_tensor(R1[:], X2S[:], X2H[:], op=OP.subtract)
    X2M = sb.tile([GPACK, SP], BF16, tag="x2m")
    nc.scalar.activation(X2M[:], R1[:], AF.Copy)
    R2 = sb.tile([GPACK, SP], F32, tag="x2r2")
    nc.vector.tensor_tensor(R2[:], R1[:], X2M[:], op=OP.subtract)
    X2L = sb.tile([GPACK, SP], BF16, tag="x2l")
    nc.scalar.activation(X2L[:], R2[:], AF.Copy)
    ONESS = sb.tile([3 * GPACK, SP], BF16, tag="oness")
    nc.gpsimd.memset(ONESS[:], 1.0)

    LH = sb.tile([K, S], BF16, tag="lh")
    for r0, src in ((0, XH), (6, XL), (12, ONESS)):
        nc.sync.dma_start(LH[r0:r0 + 3, :], src[:])
    for r0, src in ((3, XH), (9, XL), (15, ONESS)):
        nc.gpsimd.dma_start(LH[r0:r0 + 3, :], src[:])
    for r, src in ((18, X2H), (19, X2M), (20, X2L)):
        nc.gpsimd.dma_start(LH[r:r + 1, :], src[:])
    return LH


def _minloop(nc, wk, ps_a, ps_s, LH, RH, MINS, INF):
    """Per m-tile: 4 act groups of 1024 (ScalarE -> f16 CC) and 4 chained
    1024-col running-min scans (VectorE) pairing fresh PSUM with CC.
    Both PSUM tags are double-buffered (8 banks total) so each group's
    matmuls prefill while the previous group is consumed."""
    for m in range(NM):
        LHm = LH[:, m * 128:(m + 1) * 128]
        OB = wk.tile([128, 4096], F16, tag="so")
        for u in range(4):  # unit = [act 1024 | scan 1024], scans independent
            PA = ps_a.tile([128, 1024], F32, tag="pa")
            base = u * 2048
            for t in range(2):
                nc.tensor.matmul(PA[:, t * 512:(t + 1) * 512], LHm,
                                 RH[:, base + t * 512:base + (t + 1) * 512],
                                 start=True, stop=True)
            CC = wk.tile([128, 1024], F16, tag="cc")
            nc.scalar.activation(CC[:], PA[:], AF.Copy)
            PS = ps_s.tile([128, 1024], F32, tag="psc")
            for t in range(2):
                nc.tensor.matmul(PS[:, t * 512:(t + 1) * 512], LHm,
                                 RH[:, base + 1024 + t * 512:base + 1024 + (t + 1) * 512],
                                 start=True, stop=True)
            nc.vector.tensor_tensor_scan(OB[:, u * 1024:(u + 1) * 1024], PS[:], CC[:],
                                         INF[:], op0=OP.min, op1=OP.min)
        # m-tile min = min over the 4 independent scans' final columns
        nc.vector.tensor_reduce(MINS[:, m:m + 1], OB[:, 1023::1024], axis=AX.X, op=OP.min)


def _build_module():
    nc = bacc.Bacc("TRN2", target_bir_lowering=False, debug=False, num_devices=BS)
    grid_p = nc.dram_tensor("grid_p", [3 * GPACK, SP], F32, kind="ExternalInput").ap()
    sel24 = nc.dram_tensor("sel24", [3 * GPACK, GPACK], F32, kind="ExternalInput").ap()
    preds_p = nc.dram_tensor("preds_p", [3 * PACK, JP], F32, kind="ExternalInput").ap()
    gts_p = nc.dram_tensor("gts_p", [3 * PACK, JP], F32, kind="ExternalInput").ap()
    out_d = nc.dram_tensor("out", [1, 1], F32, kind="ExternalOutput").ap()

    with tile.TileContext(nc) as tc:
        with tc.tile_pool(name="sb", bufs=1) as sb, \
             tc.tile_pool(name="wk", bufs=2) as wk, \
             tc.tile_pool(name="ps_a", bufs=2, space="PSUM") as ps_a, \
             tc.tile_pool(name="ps_s", bufs=2, space="PSUM") as ps_s:
            LH = _build_lhs(nc, sb, ps_s, grid_p, sel24)
            RHP = _build_rhs(nc, sb, preds_p, "p", nc.sync.dma_start)
            RHG = _build_rhs(nc, sb, gts_p, "g", nc.gpsimd.dma_start)

            INF = sb.tile([128, 1], F32, tag="inf")
            nc.vector.memset(INF[:], 3.0e38)

            MINS_P = sb.tile([128, NM], F32, tag="minsp")
            MINS_G = sb.tile([128, NM], F32, tag="minsg")

            # d = sqrt(max(d^2, eps)) with one Newton refinement step
            def _distances(MINS, tag):
                D2 = sb.tile([128, NM], F32, tag=f"d2{tag}")
                nc.vector.tensor_scalar_max(D2[:], MINS[:], 1e-12)
                D0 = sb.tile([128, NM], F32, tag=f"d0{tag}")
                nc.scalar.activation(D0[:], D2[:], AF.Sqrt)
                R = sb.tile([128, NM], F32, tag=f"r{tag}")
                nc.vector.reciprocal(R[:], D0[:])
                D1 = sb.tile([128, NM], F32, tag=f"d1{tag}")
                nc.vector.tensor_tensor(D1[:], D2[:], R[:], op=OP.mult)
                nc.vector.tensor_tensor(D1[:], D1[:], D0[:], op=OP.add)
                nc.vector.tensor_scalar_mul(D1[:], D1[:], 0.5)
                return D1

            _minloop(nc, wk, ps_a, ps_s, LH, RHP, MINS_P, INF)
            DP = _distances(MINS_P, "dp")
            _minloop(nc, wk, ps_a, ps_s, LH, RHG, MINS_G, INF)
            DG = _distances(MINS_G, "dg")

            # mean_s |dp - dg|
            DIFF = sb.tile([128, NM], F32, tag="diff")
            nc.vector.tensor_tensor(DIFF[:], DP[:], DG[:], op=OP.subtract)
            SROW = sb.tile([128, 1], F32, tag="srow")
            nc.vector.tensor_reduce(SROW[:], DIFF[:], axis=AX.X, op=OP.add,
                                    apply_absolute_value=True)
            ONE32 = sb.tile([128, 1], F32, tag="one32")
            nc.vector.memset(ONE32[:], 1.0)
            PGX = ps_a.tile([128, 1024], F32, tag="pa")
            TOT = PGX[0:1, 0:1]
            nc.tensor.matmul(TOT, ONE32[:], SROW[:], start=True, stop=True)
            OUT = sb.tile([1, 1], F32, tag="outsb")
            nc.scalar.activation(OUT[:], TOT, AF.Copy, scale=1.0 / float(S))
            nc.sync.dma_start(out_d, OUT[:])
    nc.compile()
    return nc


_NC = None


def _get_nc():
    global _NC
    if _NC is None:
        _NC = _build_module()
    return _NC


_SEL24 = np.zeros((3 * GPACK, GPACK), np.float32)
for _r in range(3 * GPACK):
    _SEL24[_r, _r % GPACK] = 1.0


def _in_maps(gts, preds, grid_points):
    maps = []
    for b in range(BS):
        g = np.ascontiguousarray(grid_points[b], np.float32)
        maps.append({
            "grid_p": np.ascontiguousarray(g.T.reshape(3 * GPACK, SP)),
            "sel24": _SEL24,
            "preds_p": np.ascontiguousarray(preds[b], np.float32).T.reshape(3 * PACK, JP).copy(),
            "gts_p": np.ascontiguousarray(gts[b], np.float32).T.reshape(3 * PACK, JP).copy(),
        })
    return maps


def kernel(gts, preds, grid_points, _trace=False, _trace_kwargs=None):
    nc = _get_nc()
    res = bass_utils.run_bass_kernel_spmd(
        nc, _in_maps(gts, preds, grid_points), core_ids=list(range(BS)),
        trace=_trace, **(_trace_kwargs or {}))
    out = np.array([res.results[b]["out"][0, 0] for b in range(BS)], np.float32)
    if _trace:
        return out, res
    return out


# revision 9
# speedup vs baseline: 1.2909x; 1.0819x over previous
"""Chamfer-augmented kernel for Trainium2 (8 NeuronCores, data-parallel over batch).

For each batch b and each grid sample s:
    mins[s]  = min_j ||grid_s - pred_j||
    mins2[s] = min_j ||grid_s - gt_j||
    out[b]   = mean_s |mins - mins2|

Per-core algorithm (batch b on core b):
  PSUM holds d^2(s,j) = x_s^2 + q_j - 2 x_s . y_j directly: a single K=21 bf16
  matmul per 512-col chunk using exact Karatsuba splits (x = xh+xl, y' = -2y =
  yh+yl, q = qh+ql per coordinate, x^2 = x2h+x2m+x2l):
    lhsT rows: [xh]*3 [xh]*3 [xl]*3 [xl]*3 [1]*6 [x2h x2m x2l]
    rhs  rows: [yh]*3 [yl]*3 [yh]*3 [yl]*3 [qh]*3 [ql]*3 [1]*3
  Evacuation never materializes the distance matrix: per m-tile (128 samples),
  8192 columns stream through an 8-bank PSUM ring as two 2048-col groups that
  ScalarE converts to f16 (CC) and four 1024-col groups that VectorE consumes
  with fused running-min scans:
    tensor_tensor_scan(out, data0=PSUM_f32, data1=CC_f16, init=chain,
                       op0=min, op1=min)
  Each scan first-touches 1 PSUM + 1 CC element per cycle, and the chain's
  initial value threads the running min across the four scans, so the m-tile
  min falls out of the last scan's final column with no separate fold tree.
"""

import os

import numpy as np

import concourse.bass as bass
import concourse.tile as tile
from concourse import bacc, mybir, bass_utils

F32 = mybir.dt.float32
BF16 = mybir.dt.bfloat16
F16 = mybir.dt.float16
AX = mybir.AxisListType
OP = mybir.AluOpType
AF = mybir.ActivationFunctionType

BS = 8
S = 2048          # n_samples (grid points)
J = 8192          # n_points (preds/gts)
NM = S // 128     # 16 m-tiles
PACK = 8          # prep packing for rhs: [3*PACK, J/PACK]
JP = J // PACK    # 1024
GPACK = 8         # prep packing for grid: [3*GPACK, S/GPACK]
SP = S // GPACK   # 256

# lhsT/rhs row layout (K = 21)
#   rows 0-2   lhsT xh_c        rhs yh_c
#   rows 3-5   lhsT xh_c        rhs yl_c
#   rows 6-8   lhsT xl_c        rhs yh_c
#   rows 9-11  lhsT xl_c        rhs yl_c
#   rows 12-14 lhsT ones        rhs qh_c
#   rows 15-17 lhsT ones        rhs ql_c
#   rows 18-20 lhsT x2h/m/l     rhs ones
K = 21


def _build_rhs(nc, sb, pts_dram, name, dma):
    """Load one point set (packed [24, 1024] f32) and build the [21, J] bf16 rhs."""
    Y = sb.tile([3 * PACK, JP], F32, tag=f"y_{name}")
    nc.sync.dma_start(Y[:], pts_dram)
    # q = y^2 per coordinate (ScalarE), yh = bf16(-2y) (ScalarE)
    SQ = sb.tile([3 * PACK, JP], F32, tag=f"sq_{name}")
    nc.scalar.activation(SQ[:], Y[:], AF.Square)
    YH = sb.tile([3 * PACK, JP], BF16, tag=f"yh_{name}")
    nc.scalar.activation(YH[:], Y[:], AF.Copy, scale=-2.0)
    # yl = (-2y) - yh (VectorE), qh = bf16(q) (ScalarE), ql = q - qh (VectorE)
    YL = sb.tile([3 * PACK, JP], BF16, tag=f"yl_{name}")
    nc.vector.scalar_tensor_tensor(YL[:], Y[:], -2.0, YH[:], op0=OP.mult, op1=OP.subtract)
    QH = sb.tile([3 * PACK, JP], BF16, tag=f"qh_{name}")
    nc.scalar.activation(QH[:], SQ[:], AF.Copy)
    QL = sb.tile([3 * PACK, JP], BF16, tag=f"ql_{name}")
    nc.vector.tensor_tensor(QL[:], SQ[:], QH[:], op=OP.subtract)
    ONESJ = sb.tile([3 * PACK, JP], BF16, tag=f"onesj_{name}")
    nc.gpsimd.memset(ONESJ[:], 1.0)

    RH = sb.tile([K, J], BF16, tag=f"rh_{name}")
    # packed [24, 1024] -> [3, 8192] row groups; AP iteration orders match.
    for r0, src in ((0, YH), (3, YL), (6, YH), (9, YL), (12, QH), (15, QL), (18, ONESJ)):
        dma(RH[r0:r0 + 3, :], src[:])
    return RH


def _build_lhs(nc, sb, ps_s, grid_dram, sel_dram):
    """Build the [21, S] bf16 lhsT from the packed grid [24, 256]."""
    GP = sb.tile([3 * GPACK, SP], F32, tag="gp")
    nc.sync.dma_start(GP[:], grid_dram)
    SEL = sb.tile([3 * GPACK, GPACK], F32, tag="sel")
    nc.sync.dma_start(SEL[:], sel_dram)

    XH = sb.tile([3 * GPACK, SP], BF16, tag="xh")
    nc.scalar.activation(XH[:], GP[:], AF.Copy)
    XL = sb.tile([3 * GPACK, SP], BF16, tag="xl")
    nc.vector.tensor_tensor(XL[:], GP[:], XH[:], op=OP.subtract)
    SQG = sb.tile([3 * GPACK, SP], F32, tag="sqg")
    nc.scalar.activation(SQG[:], GP[:], AF.Square)
    # x^2 = sum over coords via selector matmul: [24,8].T @ [24,256] -> [8,256]
    PX = ps_s.tile([128, 1024], F32, tag="psc")
    X2P = PX[0:GPACK, 0:SP]
    nc.tensor.matmul(X2P, SEL[:], SQG[:], start=True, stop=True)
    X2S = sb.tile([GPACK, SP], F32, tag="x2s")
    nc.scalar.activation(X2S[:], X2P, AF.Copy)
    # three-term bf16 split of x^2, each packed [8, 256]
    X2H = sb.tile([GPACK, SP], BF16, tag="x2h")
    nc.scalar.activation(X2H[:], X2S[:], AF.Copy)
    R1 = sb.tile([GPACK, SP], F32, tag="x2r1")
    nc.vector.tensor_tensor(R1[:], X2S[:], X2H[:], op=OP.subtract)
    X2M = sb.tile([GPACK, SP], BF16, tag="x2m")
    nc.scalar.activation(X2M[:], R1[:], AF.Copy)
    R2 = sb.tile([GPACK, SP], F32, tag="x2r2")
    nc.vector.tensor_tensor(R2[:], R1[:], X2M[:], op=OP.subtract)
    X2L = sb.tile([GPACK, SP], BF16, tag="x2l")
    nc.scalar.activation(X2L[:], R2[:], AF.Copy)
    ONESS = sb.tile([3 * GPACK, SP], BF16, tag="oness")
    nc.gpsimd.memset(ONESS[:], 1.0)

    LH = sb.tile([K, S], BF16, tag="lh")
    for r0, src in ((0, XH), (6, XL), (12, ONESS)):
        nc.sync.dma_start(LH[r0:r0 + 3, :], src[:])
    for r0, src in ((3, XH), (9, XL), (15, ONESS)):
        nc.gpsimd.dma_start(LH[r0:r0 + 3, :], src[:])
    for r, src in ((18, X2H), (19, X2M), (20, X2L)):
        nc.gpsimd.dma_start(LH[r:r + 1, :], src[:])
    return LH


def _minloop(nc, wk, ps_a, ps_s, LH, RH, MINS, INF):
    """Per m-tile: 4 act groups of 1024 (ScalarE -> f16 CC) and 4 chained
    1024-col running-min scans (VectorE) pairing fresh PSUM with CC.
    Both PSUM tags are double-buffered (8 banks total) so each group's
    matmuls prefill while the previous group is consumed."""
    for m in range(NM):
        LHm = LH[:, m * 128:(m + 1) * 128]
        OB = wk.tile([128, 4096], F16, tag="so")
        for u in range(4):  # unit = [act 1024 | scan 1024], scans independent
            PA = ps_a.tile([128, 1024], F32, tag="pa")
            base = u * 2048
            for t in range(2):
                nc.tensor.matmul(PA[:, t * 512:(t + 1) * 512], LHm,
                                 RH[:, base + t * 512:base + (t + 1) * 512],
                                 start=True, stop=True)
            CC = wk.tile([128, 1024], F16, tag="cc", bufs=4)
            nc.scalar.activation(CC[:], PA[:], AF.Copy)
            PS = ps_s.tile([128, 1024], F32, tag="psc")
            for t in range(2):
                nc.tensor.matmul(PS[:, t * 512:(t + 1) * 512], LHm,
                                 RH[:, base + 1024 + t * 512:base + 1024 + (t + 1) * 512],
                                 start=True, stop=True)
            nc.vector.tensor_tensor_scan(OB[:, u * 1024:(u + 1) * 1024], PS[:], CC[:],
                                         INF[:], op0=OP.min, op1=OP.min)
        # m-tile min = min over the 4 independent scans' final columns
        nc.vector.tensor_reduce(MINS[:, m:m + 1], OB[:, 1023::1024], axis=AX.X, op=OP.min)


def _build_module():
    nc = bacc.Bacc("TRN2", target_bir_lowering=False, debug=False, num_devices=BS)
    grid_p = nc.dram_tensor("grid_p", [3 * GPACK, SP], F32, kind="ExternalInput").ap()
    sel24 = nc.dram_tensor("sel24", [3 * GPACK, GPACK], F32, kind="ExternalInput").ap()
    preds_p = nc.dram_tensor("preds_p", [3 * PACK, JP], F32, kind="ExternalInput").ap()
    gts_p = nc.dram_tensor("gts_p", [3 * PACK, JP], F32, kind="ExternalInput").ap()
    out_d = nc.dram_tensor("out", [1, 1], F32, kind="ExternalOutput").ap()

    with tile.TileContext(nc) as tc:
        with tc.tile_pool(name="sb", bufs=1) as sb, \
             tc.tile_pool(name="wk", bufs=2) as wk, \
             tc.tile_pool(name="ps_a", bufs=2, space="PSUM") as ps_a, \
             tc.tile_pool(name="ps_s", bufs=2, space="PSUM") as ps_s:
            LH = _build_lhs(nc, sb, ps_s, grid_p, sel24)
            RHP = _build_rhs(nc, sb, preds_p, "p", nc.sync.dma_start)
            RHG = _build_rhs(nc, sb, gts_p, "g", nc.gpsimd.dma_start)

            INF = sb.tile([128, 1], F32, tag="inf")
            nc.vector.memset(INF[:], 3.0e38)

            MINS_P = sb.tile([128, NM], F32, tag="minsp")
            MINS_G = sb.tile([128, NM], F32, tag="minsg")

            # d = sqrt(max(d^2, eps)) with one Newton refinement step
            def _distances(MINS, tag):
                D2 = sb.tile([128, NM], F32, tag=f"d2{tag}")
                nc.vector.tensor_scalar_max(D2[:], MINS[:], 1e-12)
                D0 = sb.tile([128, NM], F32, tag=f"d0{tag}")
                nc.scalar.activation(D0[:], D2[:], AF.Sqrt)
                R = sb.tile([128, NM], F32, tag=f"r{tag}")
                nc.vector.reciprocal(R[:], D0[:])
                D1 = sb.tile([128, NM], F32, tag=f"d1{tag}")
                nc.vector.tensor_tensor(D1[:], D2[:], R[:], op=OP.mult)
                nc.vector.tensor_tensor(D1[:], D1[:], D0[:], op=OP.add)
                nc.vector.tensor_scalar_mul(D1[:], D1[:], 0.5)
                return D1

            _minloop(nc, wk, ps_a, ps_s, LH, RHP, MINS_P, INF)
            DP = _distances(MINS_P, "dp")
            _minloop(nc, wk, ps_a, ps_s, LH, RHG, MINS_G, INF)
            DG = _distances(MINS_G, "dg")

            # mean_s |dp - dg|
            DIFF = sb.tile([128, NM], F32, tag="diff")
            nc.vector.tensor_tensor(DIFF[:], DP[:], DG[:], op=OP.subtract)
            SROW = sb.tile([128, 1], F32, tag="srow")
            nc.vector.tensor_reduce(SROW[:], DIFF[:], axis=AX.X, op=OP.add,
                                    apply_absolute_value=True)
            ONE32 = sb.tile([128, 1], F32, tag="one32")
            nc.vector.memset(ONE32[:], 1.0)
            PGX = ps_a.tile([128, 1024], F32, tag="pa")
            TOT = PGX[0:1, 0:1]
            nc.tensor.matmul(TOT, ONE32[:], SROW[:], start=True, stop=True)
            OUT = sb.tile([1, 1], F32, tag="outsb")
            nc.scalar.activation(OUT[:], TOT, AF.Copy, scale=1.0 / float(S))
            nc.sync.dma_start(out_d, OUT[:])
    nc.compile()
    return nc


_NC = None


def _get_nc():
    global _NC
    if _NC is None:
        _NC = _build_module()
    return _NC


_SEL24 = np.zeros((3 * GPACK, GPACK), np.float32)
for _r in range(3 * GPACK):
    _SEL24[_r, _r % GPACK] = 1.0


def _in_maps(gts, preds, grid_points):
    maps = []
    for b in range(BS):
        g = np.ascontiguousarray(grid_points[b], np.float32)
        maps.append({
            "grid_p": np.ascontiguousarray(g.T.reshape(3 * GPACK, SP)),
            "sel24": _SEL24,
            "preds_p": np.ascontiguousarray(preds[b], np.float32).T.reshape(3 * PACK, JP).copy(),
            "gts_p": np.ascontiguousarray(gts[b], np.float32).T.reshape(3 * PACK, JP).copy(),
        })
    return maps


def kernel(gts, preds, grid_points, _trace=False, _trace_kwargs=None):
    nc = _get_nc()
    res = bass_utils.run_bass_kernel_spmd(
        nc, _in_maps(gts, preds, grid_points), core_ids=list(range(BS)),
        trace=_trace, **(_trace_kwargs or {}))
    out = np.array([res.results[b]["out"][0, 0] for b in range(BS)], np.float32)
    if _trace:
        return out, res
    return out


# revision 10
# speedup vs baseline: 1.3071x; 1.0126x over previous
"""Chamfer-augmented kernel for Trainium2 (8 NeuronCores, data-parallel over batch).

For each batch b and each grid sample s:
    mins[s]  = min_j ||grid_s - pred_j||
    mins2[s] = min_j ||grid_s - gt_j||
    out[b]   = mean_s |mins - mins2|

Per-core algorithm (batch b on core b):
  PSUM holds d^2(s,j) = x_s^2 + q_j - 2 x_s . y_j directly: a single K=21 bf16
  matmul per 512-col chunk using exact Karatsuba splits (x = xh+xl, y' = -2y =
  yh+yl, q = qh+ql per coordinate, x^2 = x2h+x2m+x2l):
    lhsT rows: [xh]*3 [xh]*3 [xl]*3 [xl]*3 [1]*6 [x2h x2m x2l]
    rhs  rows: [yh]*3 [yl]*3 [yh]*3 [yl]*3 [qh]*3 [ql]*3 [1]*3
  Evacuation never materializes the distance matrix: per m-tile (128 samples),
  8192 columns stream through an 8-bank PSUM ring as two 2048-col groups that
  ScalarE converts to f16 (CC) and four 1024-col groups that VectorE consumes
  with fused running-min scans:
    tensor_tensor_scan(out, data0=PSUM_f32, data1=CC_f16, init=chain,
                       op0=min, op1=min)
  Each scan first-touches 1 PSUM + 1 CC element per cycle, and the chain's
  initial value threads the running min across the four scans, so the m-tile
  min falls out of the last scan's final column with no separate fold tree.
"""

import os

import numpy as np

import concourse.bass as bass
import concourse.tile as tile
from concourse import bacc, mybir, bass_utils

F32 = mybir.dt.float32
BF16 = mybir.dt.bfloat16
F16 = mybir.dt.float16
AX = mybir.AxisListType
OP = mybir.AluOpType
AF = mybir.ActivationFunctionType

BS = 8
S = 2048          # n_samples (grid points)
J = 8192          # n_points (preds/gts)
NM = S // 128     # 16 m-tiles
PACK = 8          # prep packing for rhs: [3*PACK, J/PACK]
JP = J // PACK    # 1024
GPACK = 8         # prep packing for grid: [3*GPACK, S/GPACK]
SP = S // GPACK   # 256

# lhsT/rhs row layout (K = 24)
#   rows 0-2   lhsT xh_c        rhs yh_c
#   rows 3-5   lhsT xh_c        rhs yl_c
#   rows 6-8   lhsT xl_c        rhs yh_c
#   rows 9-11  lhsT xl_c        rhs yl_c
#   rows 12-14 lhsT ones        rhs qh_c
#   rows 15-17 lhsT ones        rhs ql_c
#   rows 18-23 lhsT gqh_c/gql_c rhs ones     (x^2 = sum_c g_c^2 via contraction)
K = 24


def _build_rhs(nc, sb, pts_dram, name, dma):
    """Load one point set (packed [24, 1024] f32) and build the [21, J] bf16 rhs."""
    Y = sb.tile([3 * PACK, JP], F32, tag=f"y_{name}")
    nc.sync.dma_start(Y[:], pts_dram)
    # q = y^2 per coordinate (ScalarE), yh = bf16(-2y) (ScalarE)
    SQ = sb.tile([3 * PACK, JP], F32, tag=f"sq_{name}")
    nc.scalar.activation(SQ[:], Y[:], AF.Square)
    YH = sb.tile([3 * PACK, JP], BF16, tag=f"yh_{name}")
    nc.scalar.activation(YH[:], Y[:], AF.Copy, scale=-2.0)
    # yl = (-2y) - yh (VectorE), qh = bf16(q) (ScalarE), ql = q - qh (VectorE)
    YL = sb.tile([3 * PACK, JP], BF16, tag=f"yl_{name}")
    nc.vector.scalar_tensor_tensor(YL[:], Y[:], -2.0, YH[:], op0=OP.mult, op1=OP.subtract)
    QH = sb.tile([3 * PACK, JP], BF16, tag=f"qh_{name}")
    nc.scalar.activation(QH[:], SQ[:], AF.Copy)
    QL = sb.tile([3 * PACK, JP], BF16, tag=f"ql_{name}")
    nc.vector.tensor_tensor(QL[:], SQ[:], QH[:], op=OP.subtract)
    ONESJ = sb.tile([3 * PACK, JP], BF16, tag=f"onesj_{name}")
    nc.gpsimd.memset(ONESJ[:], 1.0)

    RH = sb.tile([K, J], BF16, tag=f"rh_{name}")
    # packed [24, 1024] -> [3, 8192] row groups; AP iteration orders match.
    # ScalarE-sourced rows first so VectorE-dependent rows don't head-of-line
    # block the in-order DGE queue.
    for r0, src in ((18, ONESJ), (21, ONESJ), (0, YH), (6, YH), (12, QH),
                    (3, YL), (9, YL), (15, QL)):
        dma(RH[r0:r0 + 3, :], src[:])
    return RH


def _build_lhs(nc, sb, grid_dram):
    """Build the [24, S] bf16 lhsT from the packed grid [24, 256]."""
    GP = sb.tile([3 * GPACK, SP], F32, tag="gp")
    nc.sync.dma_start(GP[:], grid_dram)

    XH = sb.tile([3 * GPACK, SP], BF16, tag="xh")
    nc.scalar.activation(XH[:], GP[:], AF.Copy)
    XL = sb.tile([3 * GPACK, SP], BF16, tag="xl")
    nc.vector.tensor_tensor(XL[:], GP[:], XH[:], op=OP.subtract)
    # per-coord squares of the grid, split to bf16 pairs (x^2 via contraction)
    SQG = sb.tile([3 * GPACK, SP], F32, tag="sqg")
    nc.vector.tensor_tensor(SQG[:], GP[:], GP[:], op=OP.mult)
    GQH = sb.tile([3 * GPACK, SP], BF16, tag="gqh")
    nc.scalar.activation(GQH[:], SQG[:], AF.Copy)
    GQL = sb.tile([3 * GPACK, SP], BF16, tag="gql")
    nc.vector.tensor_tensor(GQL[:], SQG[:], GQH[:], op=OP.subtract)
    ONESS = sb.tile([3 * GPACK, SP], BF16, tag="oness")
    nc.gpsimd.memset(ONESS[:], 1.0)

    LH = sb.tile([K, S], BF16, tag="lh")
    for r0, src in ((0, XH), (6, XL), (12, ONESS), (18, GQH)):
        nc.sync.dma_start(LH[r0:r0 + 3, :], src[:])
    for r0, src in ((3, XH), (9, XL), (15, ONESS), (21, GQL)):
        nc.gpsimd.dma_start(LH[r0:r0 + 3, :], src[:])
    return LH


def _minloop(nc, wk, ps_a, ps_s, LH, RH, MINS, INF):
    """Per m-tile: 4 act groups of 1024 (ScalarE -> f16 CC) and 4 chained
    1024-col running-min scans (VectorE) pairing fresh PSUM with CC.
    Both PSUM tags are double-buffered (8 banks total) so each group's
    matmuls prefill while the previous group is consumed."""
    for m in range(NM):
        LHm = LH[:, m * 128:(m + 1) * 128]
        OB = wk.tile([128, 4096], F16, tag="so")
        for u in range(4):  # unit = [act 1024 | scan 1024], scans independent
            PA = ps_a.tile([128, 1024], F32, tag="pa")
            base = u * 2048
            for t in range(2):
                nc.tensor.matmul(PA[:, t * 512:(t + 1) * 512], LHm,
                                 RH[:, base + t * 512:base + (t + 1) * 512],
                                 start=True, stop=True)
            CC = wk.tile([128, 1024], F16, tag="cc", bufs=4)
            nc.scalar.activation(CC[:], PA[:], AF.Copy)
            PS = ps_s.tile([128, 1024], F32, tag="psc")
            for t in range(2):
                nc.tensor.matmul(PS[:, t * 512:(t + 1) * 512], LHm,
                                 RH[:, base + 1024 + t * 512:base + 1024 + (t + 1) * 512],
                                 start=True, stop=True)
            nc.vector.tensor_tensor_scan(OB[:, u * 1024:(u + 1) * 1024], PS[:], CC[:],
                                         INF[:], op0=OP.min, op1=OP.min)
        # m-tile min = min over the 4 independent scans' final columns
        nc.vector.tensor_reduce(MINS[:, m:m + 1], OB[:, 1023::1024], axis=AX.X, op=OP.min)


def _build_module():
    nc = bacc.Bacc("TRN2", target_bir_lowering=False, debug=False, num_devices=BS)
    grid_p = nc.dram_tensor("grid_p", [3 * GPACK, SP], F32, kind="ExternalInput").ap()
    preds_p = nc.dram_tensor("preds_p", [3 * PACK, JP], F32, kind="ExternalInput").ap()
    gts_p = nc.dram_tensor("gts_p", [3 * PACK, JP], F32, kind="ExternalInput").ap()
    out_d = nc.dram_tensor("out", [1, 1], F32, kind="ExternalOutput").ap()

    with tile.TileContext(nc) as tc:
        with tc.tile_pool(name="sb", bufs=1) as sb, \
             tc.tile_pool(name="wk", bufs=2) as wk, \
             tc.tile_pool(name="ps_a", bufs=2, space="PSUM") as ps_a, \
             tc.tile_pool(name="ps_s", bufs=2, space="PSUM") as ps_s:
            LH = _build_lhs(nc, sb, grid_p)
            RHP = _build_rhs(nc, sb, preds_p, "p", nc.sync.dma_start)
            RHG = _build_rhs(nc, sb, gts_p, "g", nc.gpsimd.dma_start)

            INF = sb.tile([128, 1], F32, tag="inf")
            nc.vector.memset(INF[:], 3.0e38)

            MINS_P = sb.tile([128, NM], F32, tag="minsp")
            MINS_G = sb.tile([128, NM], F32, tag="minsg")

            # d = sqrt(max(d^2, eps)) with one Newton refinement step
            def _distances(MINS, tag):
                D2 = sb.tile([128, NM], F32, tag=f"d2{tag}")
                nc.vector.tensor_scalar_max(D2[:], MINS[:], 1e-12)
                D0 = sb.tile([128, NM], F32, tag=f"d0{tag}")
                nc.scalar.activation(D0[:], D2[:], AF.Sqrt)
                R = sb.tile([128, NM], F32, tag=f"r{tag}")
                nc.vector.reciprocal(R[:], D0[:])
                D1 = sb.tile([128, NM], F32, tag=f"d1{tag}")
                nc.vector.tensor_tensor(D1[:], D2[:], R[:], op=OP.mult)
                nc.vector.tensor_tensor(D1[:], D1[:], D0[:], op=OP.add)
                nc.vector.tensor_scalar_mul(D1[:], D1[:], 0.5)
                return D1

            _minloop(nc, wk, ps_a, ps_s, LH, RHP, MINS_P, INF)
            DP = _distances(MINS_P, "dp")
            _minloop(nc, wk, ps_a, ps_s, LH, RHG, MINS_G, INF)
            DG = _distances(MINS_G, "dg")

            # mean_s |dp - dg|
            DIFF = sb.tile([128, NM], F32, tag="diff")
            nc.vector.tensor_tensor(DIFF[:], DP[:], DG[:], op=OP.subtract)
            SROW = sb.tile([128, 1], F32, tag="srow")
            nc.vector.tensor_reduce(SROW[:], DIFF[:], axis=AX.X, op=OP.add,
                                    apply_absolute_value=True)
            ONE32 = sb.tile([128, 1], F32, tag="one32")
            nc.vector.memset(ONE32[:], 1.0)
            PGX = ps_a.tile([128, 1024], F32, tag="pa")
            TOT = PGX[0:1, 0:1]
            nc.tensor.matmul(TOT, ONE32[:], SROW[:], start=True, stop=True)
            OUT = sb.tile([1, 1], F32, tag="outsb")
            nc.scalar.activation(OUT[:], TOT, AF.Copy, scale=1.0 / float(S))
            nc.sync.dma_start(out_d, OUT[:])
    nc.compile()
    return nc


_NC = None


def _get_nc():
    global _NC
    if _NC is None:
        _NC = _build_module()
    return _NC


def _in_maps(gts, preds, grid_points):
    maps = []
    for b in range(BS):
        g = np.ascontiguousarray(grid_points[b], np.float32)
        maps.append({
            "grid_p": np.ascontiguousarray(g.T.reshape(3 * GPACK, SP)),
            "preds_p": np.ascontiguousarray(preds[b], np.float32).T.reshape(3 * PACK, JP).copy(),
            "gts_p": np.ascontiguousarray(gts[b], np.float32).T.reshape(3 * PACK, JP).copy(),
        })
    return maps


def kernel(gts, preds, grid_points, _trace=False, _trace_kwargs=None):
    nc = _get_nc()
    res = bass_utils.run_bass_kernel_spmd(
        nc, _in_maps(gts, preds, grid_points), core_ids=list(range(BS)),
        trace=_trace, **(_trace_kwargs or {}))
    out = np.array([res.results[b]["out"][0, 0] for b in range(BS)], np.float32)
    if _trace:
        return out, res
    return out
